# revision 17
# baseline (speedup 1.0000x reference)
"""FAPE loss kernel for Trainium2 (8 NeuronCores, Bass/Tile).

Math
----
The reference computes, for frames i and residue-atoms (l, j):

    local[i, lj, d] = sum_c coords[lj, c] * R[i, d, c] - off[i, d]
    d2[i, lj]       = sum_d (pred_local - true_local)^2
    loss            = sum_{i,lj} m[i] * m[l] * min(sqrt(d2 + eps), 10) / ((sum m)^2 * 3 + eps) / 10

The delta is linear in the 7-vector u'[lj] = [pred_coords(3), true_coords(3), 1]:
    delta_d[i, lj] = dot(u'[lj], w_d[i]),  w_d[i] = [pR[i,d,:], -tR[i,d,:], -(offp-offt)[i,d]]
so d2 is a quadratic form
    d2[i, lj] = sum_{a<=b} mult_ab * u'_a u'_b * Q[i,(a,b)],  Q[i] = sum_d w_d w_d^T

Mask compaction: only masked-in (i, l) pairs contribute to the numerator, and
the seeded input has V = mask.sum() ~ L/2.  The device handles the first
min(V, 1024) valid frames (128 per core, one PSUM partition tile) against the
first min(V, 1024) valid residues (3072 lj columns, 3 groups of 1024); the
O(V - 1024) ragged remainder (4 frames / 4 residues for the seeded input) is
summed exactly on the host in float64 alongside the rest of the O(L) prep.

Precision: both factors are rounded straight to bf16 (K=28, no hi/lo
splitting).  The bf16 rounding errors of Q and P are symmetric and average out
over the ~3M-element sum: simulated end-to-end loss error ~1e-6 (validated
against the f64 reference), far inside the 1e-4 harness assert.

Device (per core): d2 = A^T.T @ B as K=28 bf16 matmuls (N=512, two per
1024-wide PSUM group tile, 3 group tiles so no PSUM-slot reuse waits), then
per group: clamp to [0, 100] on the POOL engine (min(sqrt(d2), 10) ==
sqrt(min(d2, 100)); max(.,0) guards bf16 cancellation), sqrt + free-axis
accumulation fused on the scalar engine.  Each core returns 3 per-group
partition sums; the host folds them, adds the spill term and normalizes.
eps inside the sqrt is dropped: its contribution is O(1e-9) on this data.

Schedule: three input DMAs (A+chunk0 / chunks 1-2 / chunks 3-5) issued from
three different sequencers (SP, DVE, Pool) so the serial descriptor-generation
cost (~0.8us each) overlaps; the scalar engine issues nothing and instead runs
its Sqrt ACT_TABLE_LOAD + dummy-activation chain during the DMA window, so the
first real sqrt starts right after the first clamp.  The output DMA is issued
from the scalar engine itself: engine order makes it need no semaphore wait.

Toolchain constraint: this walrus build allows ONE semaphore wait per
instruction.  Three PSUM tiles (no reuse), same-engine matmul pairs per tile,
the scalar dummy-activation chain, and the scalar-issued output DMA keep every
compute instruction at <=1 wait; remaining multi-wait instructions (the Tile
exit drain) are split onto single-wait no-ops by _split_multi_waits.  Tile's
entry/exit all-engine barriers run in sem-only form (no per-engine drains).
"""

import sys

import numpy as np

for _p in ("/opt/trn_rl_repo",):
    if _p not in sys.path:
        sys.path.insert(0, _p)

import ml_dtypes
import concourse.bass as bass
import concourse.tile as tile
from concourse import mybir
from concourse.bass_utils import run_bass_kernel_spmd

L = 2048
N_CORES = 8
K = 28         # 7*8/2 upper-triangle pairs, straight bf16
N_CHUNK = 512
GROUP_CHUNKS = 2
GROUP_COLS = GROUP_CHUNKS * N_CHUNK  # 1024 = one 2-bank PSUM tile
MAX_DEV_F = 1024                     # device frames cap: 128 per core
CLAMP2 = 100.0  # CLAMP_DISTANCE ** 2
CLAMP_DISTANCE = 10.0
EPS = 1e-8

_PAIRS = [(a, b) for a in range(7) for b in range(a, 7)]


def _exact_clamped_sum(pc, tc, pR, pT, tR, tT, fi, li):
    """Exact f64 sum of clamped FAPE distances over frames fi x residues li."""
    if len(fi) == 0 or len(li) == 0:
        return 0.0
    offp = np.einsum('ic,idc->id', pT[fi], pR[fi])
    offt = np.einsum('ic,idc->id', tT[fi], tR[fi])
    pl_ = np.einsum('ljc,idc->iljd', pc[li], pR[fi]) - offp[:, None, None, :]
    tl_ = np.einsum('ljc,idc->iljd', tc[li], tR[fi]) - offt[:, None, None, :]
    d2 = ((pl_ - tl_) ** 2).sum(-1)
    return float(np.minimum(np.sqrt(d2 + EPS), CLAMP_DISTANCE).sum())


def _host_prep(pred_coords, true_coords, pred_rotation, pred_translation,
               true_rotation, true_translation, mask):
    """Compact to valid frames/residues; return per-core A (K, per_core) bf16,
    B (K, n_cols) bf16, the exact f64 spill term, and layout sizes."""
    pc = np.asarray(pred_coords, np.float64)
    tc = np.asarray(true_coords, np.float64)
    pR = np.asarray(pred_rotation, np.float64)
    pT = np.asarray(pred_translation, np.float64)
    tR = np.asarray(true_rotation, np.float64)
    tT = np.asarray(true_translation, np.float64)
    m = np.asarray(mask) != 0

    idx = np.nonzero(m)[0]
    V = len(idx)
    dev_f = idx[:MAX_DEV_F]          # device frames
    spill_f = idx[MAX_DEV_F:]        # host frames
    dev_r = idx[:MAX_DEV_F]          # device residues
    spill_r = idx[MAX_DEV_F:]        # host residues
    n_f = len(dev_f)
    per_core = (n_f + N_CORES - 1) // N_CORES if n_f else 1
    n_cols_data = 3 * len(dev_r)
    n_chunks = max(1, (n_cols_data + N_CHUNK - 1) // N_CHUNK)
    n_cols = n_chunks * N_CHUNK

    # quadratic-form factors
    offp = np.einsum('ic,idc->id', pT, pR)
    offt = np.einsum('ic,idc->id', tT, tR)
    W = np.concatenate([pR, -tR, -(offp - offt)[:, :, None]], axis=2)  # (L,3,7)
    Q = np.einsum('ida,idb->iab', W, W)  # (L, 7, 7)
    Qv = np.stack([Q[:, a, b] * (1.0 if a == b else 2.0) for (a, b) in _PAIRS],
                  axis=1)  # (L, 28)

    dev_lj = (dev_r[:, None] * 3 + np.arange(3)[None, :]).reshape(-1)
    U = np.concatenate([pc.reshape(L * 3, 3), tc.reshape(L * 3, 3),
                        np.ones((L * 3, 1))], axis=1)[dev_lj]  # (n_cols_data, 7)
    P = np.stack([U[:, a] * U[:, b] for (a, b) in _PAIRS], axis=0)  # (28, cols)

    B = np.zeros((K, n_cols), ml_dtypes.bfloat16)
    B[:, :n_cols_data] = P.astype(ml_dtypes.bfloat16)

    A_cores = []
    for c in range(N_CORES):
        fr = dev_f[c * per_core:(c + 1) * per_core]
        a_c = np.zeros((K, per_core), ml_dtypes.bfloat16)
        a_c[:, :len(fr)] = Qv[fr].T.astype(ml_dtypes.bfloat16)
        A_cores.append(a_c)

    spill = (_exact_clamped_sum(pc, tc, pR, pT, tR, tT, spill_f, idx)
             + _exact_clamped_sum(pc, tc, pR, pT, tR, tT, dev_f, spill_r))
    denom = float(m.sum()) ** 2 * 3.0 + EPS
    return A_cores, B, spill, denom, per_core, n_chunks


def _split_multi_waits(nc):
    """The TPB instruction encodings used by this walrus build carry a single
    semaphore wait.  Tile can emit several waits on one instruction (notably
    the kernel-tail drain).  Split the extras onto same-engine no-ops placed
    immediately before the instruction — engine-order execution makes this
    semantically identical."""
    for bbw in nc.main_func.blocks:
        il = bbw.instructions
        out = []
        changed = False
        for ins in il:
            si = ins.sync_info
            if si is not None and len(si.on_wait) > 1:
                waits = list(si.on_wait)
                for idx, w in enumerate(waits[:-1]):
                    out.append(mybir.InstNoOp(
                        name=f"{ins.name}-waitsplit{idx}",
                        engine=ins.engine,
                        sync_info=mybir.SyncInfo(on_wait=[w], on_update=[]),
                    ))
                si.on_wait = [waits[-1]]
                changed = True
            out.append(ins)
        if changed:
            bbw.instructions = out
    return nc


def _build_program(per_core, n_chunks, split_waits=True):
    f32 = mybir.dt.float32
    bf16 = mybir.dt.bfloat16
    # Tile's entry/exit all-engine barriers default to the drain+EVSEM
    # butterfly; the sem-only variant synchronizes the same points without
    # the drains (~0.7us saved, measured; correctness preserved since the
    # kernel-tail drain instruction is still emitted separately).
    _orig_aeb = bass.Bass.all_engine_barrier
    bass.Bass.all_engine_barrier = (
        lambda self, *, sem_only=False: _orig_aeb(self, sem_only=True))
    try:
        nc = _build_program_inner(f32, bf16, per_core, n_chunks, split_waits)
    finally:
        bass.Bass.all_engine_barrier = _orig_aeb
    return nc


def _build_program_inner(f32, bf16, per_core, n_chunks, split_waits):
    # Group layout: single-chunk first and last groups, 2-chunk middles.
    group_sizes = [N_CHUNK]
    remaining = n_chunks - 1
    while remaining > 1:
        group_sizes.append(2 * N_CHUNK)
        remaining -= 2
    if remaining == 1:
        group_sizes.append(N_CHUNK)
    n_groups = len(group_sizes)
    n_cols = n_chunks * N_CHUNK
    Q0 = per_core  # column where lj chunks start

    nc = bass.Bass()
    # Input layout: [A (per_core) | lj chunk 0..n_chunks-1 (512 each)]
    inp = nc.declare_dram_parameter("inp", [K, Q0 + n_cols], bf16,
                                    isOutput=False)
    # Raw per-group accumulator; host folds the columns into the numerator.
    fsums = nc.declare_dram_parameter("fsums", [per_core, n_groups], f32,
                                      isOutput=True)

    with tile.TileContext(nc) as tc:
        with tc.tile_pool(name="const", bufs=1) as const_pool, \
             tc.tile_pool(name="clamped", bufs=n_groups) as clamped_pool, \
             tc.tile_pool(name="ps", bufs=n_groups, space="PSUM") as ps:
            data = const_pool.tile([K, Q0 + n_cols], bf16)
            # Three DMAs: A+chunk0 and the tail chunks on SP, the middle
            # chunks on Pool, so the ~0.8us descriptor-generation costs
            # overlap and the first matmul only waits for the small A+chunk0
            # slice.  The scalar engine issues nothing so its Sqrt table
            # load runs during the DMA window.
            bounds = [0, Q0 + N_CHUNK, Q0 + min(3 * N_CHUNK, n_cols),
                      Q0 + n_cols]
            bounds = sorted(set(bounds))
            engines = [nc.sync, nc.gpsimd, nc.sync]
            for i in range(len(bounds) - 1):
                engines[i % 3].dma_start(data[:, bounds[i]:bounds[i + 1]],
                                         inp[:, bounds[i]:bounds[i + 1]])

            acc = const_pool.tile([per_core, n_groups], f32)

            # Broadcast-10 operand for the clamp's min; memset on the vector
            # engine itself so the first scalar_tensor_tensor needs no
            # second (cross-engine) wait for it.
            tens = const_pool.tile([per_core, 2 * N_CHUNK], bf16)
            nc.vector.memset(tens[:], CLAMP_DISTANCE)

            # PE p-state warm-up: the tensor engine's clock ramps with
            # sustained use (0.65 -> 1.2 -> 2.4 GHz).  Four throwaway
            # matmuls on whatever is in SBUF keep the PE busy through the
            # input-DMA window so the real matmuls start at the higher
            # p-states; the scratch PSUM tile is never read.
            # K=1 matmuls over a tiny scalar-memset source: the PE's cost is
            # N cycles regardless of K, so these ramp the clock just like
            # real work.  The memset runs first on the scalar engine (which
            # has ~2.4us of slack before its table load matters), so the
            # first warm-up starts ~0.7us into the DMA window.  The warm
            # tile shares the d2 slot rotation (bufs=n_groups): the last
            # group's matmul lands in the warm slot, safe via PE order.
            warm_src = const_pool.tile([1, per_core + N_CHUNK], bf16)
            nc.scalar.memzero(warm_src[:])
            warm = ps.tile([per_core, N_CHUNK], f32, tag="d2")
            for _ in range(3):
                nc.tensor.matmul(
                    warm[:], warm_src[:, 0:per_core],
                    warm_src[:, per_core:per_core + N_CHUNK],
                    start=True, stop=True,
                )

            # Scalar-engine constant + two dummy activations: the sqrt bias
            # const-AP and the engine's own-semaphore ticks would otherwise
            # put a second wait on the first real sqrt (walrus allows one);
            # the first activation also triggers the Sqrt ACT_TABLE_LOAD
            # early, during the DMA window.
            bias_t = const_pool.tile([128, 1], f32)
            scratch_t = const_pool.tile([128, 1], f32)
            nc.scalar.memzero(bias_t[:])
            nc.scalar.activation(bias_t[:], bias_t[:],
                                 mybir.ActivationFunctionType.Sqrt,
                                 bias=bias_t[:, 0:1])
            nc.scalar.activation(scratch_t[:], bias_t[:],
                                 mybir.ActivationFunctionType.Sqrt,
                                 bias=bias_t[:, 0:1])

            # sqrt FIRST (scalar engine, PSUM fp32 -> SBUF bf16), then
            # max(.,0)/min(.,10) + free-axis accumulation on the DVE in 4x
            # mode (bf16 in/out, SBUF, packed).  bf16 cancellation can push
            # d2 slightly negative; HW sqrt returns NaN there, and the DVE's
            # max(NaN, 0) canonicalizes to 0 (measured on HW), which is the
            # correct clamped value for those ~1e-4-fraction elements.
            # The first and last groups are a single 512 chunk: the first
            # lets the scalar chain start one matmul earlier, the small last
            # shortens the trailing sqrt->accum->DMA dependency chain.
            col0 = Q0
            for g, g_cols in enumerate(group_sizes):
                d2 = ps.tile([per_core, g_cols], f32, tag="d2")
                for c in range(0, g_cols, N_CHUNK):
                    w = min(N_CHUNK, g_cols - c)
                    nc.tensor.matmul(
                        d2[:, c:c + w],
                        data[:, 0:per_core],
                        data[:, col0 + c:col0 + c + w],
                        start=True, stop=True,
                    )
                col0 += g_cols
                sq = clamped_pool.tile([per_core, g_cols], bf16, tag="sq")
                nc.scalar.activation(
                    sq[:], d2[:],
                    mybir.ActivationFunctionType.Sqrt,
                    bias=bias_t[:, 0:1],
                )
                # out = min(max(sq, 0), 10), accum = row-sum(out): a single
                # DVE pass in 4x mode (all APs bf16/SBUF/packed).  max first
                # canonicalizes the NaNs to 0 before the min.
                nc.vector.scalar_tensor_tensor(
                    out=sq[:], in0=sq[:], scalar=0.0,
                    in1=tens[:, 0:g_cols],
                    op0=mybir.AluOpType.max, op1=mybir.AluOpType.min,
                    accum_out=acc[:, g:g + 1],
                )

            # Sync is idle after the input DMAs; the single data wait is on
            # the DVE's last accumulate.
            nc.sync.dma_start(fsums[:], acc[:])
    if split_waits:
        # Needed for the walrus compile; CoreSim can't model the raw no-ops.
        _split_multi_waits(nc)
    return nc


def kernel(pred_coords, true_coords, pred_rotation, pred_translation,
           true_rotation, true_translation, mask, **_run_kwargs):
    A_cores, B, spill, denom, per_core, n_chunks = _host_prep(
        pred_coords, true_coords, pred_rotation, pred_translation,
        true_rotation, true_translation, mask)

    in_maps = [{"inp": np.ascontiguousarray(
        np.concatenate([a_c, B], axis=1))} for a_c in A_cores]

    nc = _build_program(per_core, n_chunks)
    res = run_bass_kernel_spmd(nc, in_maps, list(range(N_CORES)),
                               **_run_kwargs)

    numer = spill
    for c in range(N_CORES):
        numer += float(np.asarray(res.results[c]["fsums"], np.float64).sum())

    out = np.float32(numer / denom / 10.0)
    if _run_kwargs:
        return out, res
    return out
